# revision 11
# baseline (speedup 1.0000x reference)
"""AutoAdaptiveFocalLossV2 on 8 Trainium2 NeuronCores.

Math per row r of input [N, C]:
    s      = sum_c exp(x[r, c])
    logpt  = x[r, target[r]] - log(s)
    pt     = exp(logpt)
    gamma  = g[searchsorted(edges, pt)]
    loss_r = -(1 - pt + 1e-20)^gamma * logpt
Output = sum_r loss_r.  Sharding: pure data-parallel, 16384 rows/core.

The rel-err budget (2e-2) allows low-precision streaming. Row-blocks of
128 rows are distributed over three engines (the row-sum is the only
per-element work):
  - A-blocks (ACT): streamed as fp8_e4m3 (1 B/elem; ACT is rate-
    dtype-independent); one activation(Exp, accum_out) per block does
    exp+row-sum (~1.41 us measured).
  - D-blocks (DVE): streamed as bf16; a Schraudolph exp2 --
    tensor_scalar computes int16(x*(2^7*log2e) + (127*2^7 + adj)) whose
    bit pattern IS bf16(exp(x)) to +-3% (sawtooth centered by adj;
    total-loss error ~1e-5); the int16 output engages the DVE 4x perf
    mode (~0.34 us/block measured). A tensor_reduce row-sums it
    (~1.15 us, the DVE's fixed 1x rate -- the system bottleneck).
  - H-blocks: the same Schraudolph tensor_scalar but running on the
    otherwise-idle GpSimd/Pool engine, IN PLACE on the stream tile
    (bf16 in, int16 bits out over the same bytes -- avoiding a second
    buffer also keeps every op at a single semaphore wait); DVE then
    row-sums the tile like a D-block.
The target logit x[r, t[r]] is gathered on the host (index-driven data
movement, like the resharding) and DMA'd as a tiny fp32 side tensor.

ACT and DVE write row-sums into separate staging tiles; the host
permutes row-blocks so staging position and the xt layout agree. The
epilogue merge is 2 contiguous ACT copies per half. Streams are
engine-homogeneous (fp8 read only by ACT; bf16 touched by Pool/DVE
with DVE last), so each slot-recycling DMA carries a single wait on
the last reader engine; no joiner ops. The first epilogue half
overlaps the second half of the stream, and its 14-bin gamma mask loop
runs on Pool; the tail half keeps gamma on DVE (shorter critical
path). Per-core output is a [128] partial sum; host sums in f64.

The single-sync-wait workarounds (sink ops, post-pass wait rewrites,
the in-place H trick) exist because this walrus build refuses any
instruction carrying more than one semaphore wait;
verify_single_wait() enforces it at build time.
"""

import os
import numpy as np

N = 131072
C = 1000
NUM_BINS = 15
P = 128
NCORES = 8
RPC = N // NCORES          # 16384 rows per core
COLS = RPC // P            # 128 row-blocks per core
J = 4                      # row-blocks per chunk
CHUNKS = COLS // J         # 32 chunks per core

# Engine split: A = ACT exp+accum (fp8), D = DVE schraudolph + reduce,
# H = Pool schraudolph (in place) + DVE reduce.
N_A, N_D, N_H = 59, 5, 64

# Schraudolph exp2-in-bf16-bits constants.
SCH_C1 = 1.4426950408889634 * 128.0            # log2(e) * 2^7
SCH_C2 = 127.0 * 128.0 - 7.4                   # bias + sawtooth centering

LAST_RESULT = None


def build_assignment():
    """Per stream-column engine assignment (largest-remainder interleave),
    with each chunk's bf16 (D/H) blocks reordered D-first (a D tensor_scalar
    is a cheap DVE absorber for the chunk's DMA wait)."""
    quota = {"A": N_A, "D": N_D, "H": N_H}
    placed = {"A": 0, "D": 0, "H": 0}
    assign = []
    for c in range(COLS):
        e = max("ADH", key=lambda e: (quota[e] * (c + 1) - placed[e] * COLS,
                                      quota[e]))
        assign.append(e)
        placed[e] += 1
    out = []
    for k in range(CHUNKS):
        ch = assign[k * J : (k + 1) * J]
        dh = [e for e in ch if e != "A"]
        if "D" in dh:
            dh.remove("D")
            dh = ["D"] + dh
        a_ct = ch.count("A")
        merged = []
        ai, di = 0, 0
        order = "BABA" if a_ct < len(dh) or a_ct == 0 else "BAAB"
        for t in order:
            if t == "A" and ai < a_ct:
                merged.append("A"); ai += 1
            elif t == "B" and di < len(dh):
                merged.append(dh[di]); di += 1
        while di < len(dh):
            merged.append(dh[di]); di += 1
        while ai < a_ct:
            merged.append("A"); ai += 1
        out.extend(merged)
    assert len(out) == COLS
    assert out.count("A") == N_A and out.count("D") == N_D
    return out


ASSIGN = build_assignment()
# staging region of each column: "A" (ACT accum) or "V" (DVE reduce)
REGION = ["A" if e == "A" else "V" for e in ASSIGN]
ENG_IDX = []
_c = {"A": 0, "V": 0}
for _r in REGION:
    ENG_IDX.append(_c[_r])
    _c[_r] += 1
HALF_CT = []
for h in range(2):
    seg = REGION[h * COLS // 2 : (h + 1) * COLS // 2]
    HALF_CT.append({r: seg.count(r) for r in "AV"})


def epilogue_col(c):
    """Epilogue/staging position of stream column c inside s_all space
    (h*64 + offset): [A-region][V-region] per half."""
    r, m = REGION[c], ENG_IDX[c]
    h = 0 if c < COLS // 2 else 1
    m_in_half = m - (HALF_CT[0][r] if h else 0)
    base = 0 if r == "A" else HALF_CT[h]["A"]
    return h * (COLS // 2) + base + m_in_half


def build_program(bin_edges, bin_gammas, hw_fixups=True):
    import concourse.bass as bass
    import concourse.mybir as mybir
    import concourse.tile as tile

    f32 = mybir.dt.float32
    bf16 = mybir.dt.bfloat16
    fp8 = mybir.dt.float8e4
    i16 = mybir.dt.int16
    Alu = mybir.AluOpType
    Act = mybir.ActivationFunctionType

    edges = [float(v) for v in np.asarray(bin_edges, np.float64)]
    gammas = [float(v) for v in np.asarray(bin_gammas, np.float64)]
    assert len(edges) == NUM_BINS - 1 and len(gammas) == NUM_BINS

    nc = bass.Bass()
    x8_d = nc.dram_tensor("x8", [N_A, P, C], fp8, kind="ExternalInput")
    x16_d = nc.dram_tensor("x16", [N_D + N_H, P, C], bf16, kind="ExternalInput")
    xt_d = nc.dram_tensor("xt", [P, COLS], f32, kind="ExternalInput")
    out_d = nc.dram_tensor("out", [P, 1], f32, kind="ExternalOutput")

    a_start = [0] * (CHUNKS + 1)
    b_start = [0] * (CHUNKS + 1)
    for k in range(CHUNKS):
        ch = ASSIGN[k * J : (k + 1) * J]
        a_start[k + 1] = a_start[k] + ch.count("A")
        b_start[k + 1] = b_start[k] + J - ch.count("A")

    with tile.TileContext(nc) as tc:
        with (
            tc.tile_pool(name="x8pool", bufs=8) as x8pool,
            tc.tile_pool(name="x16pool", bufs=8) as x16pool,
            tc.tile_pool(name="scratch", bufs=2) as scratch,
            tc.tile_pool(name="epool", bufs=2) as epool,
            tc.tile_pool(name="ipoolD", bufs=2) as ipoolD,
            tc.tile_pool(name="consts", bufs=1) as consts,
            tc.tile_pool(name="stage", bufs=1) as stage,
        ):
            ones = consts.tile([P, 1], f32, tag="ones")
            nc.vector.memset(ones[:], 1.0)

            s_acc = stage.tile([P, N_A], f32, tag="s_acc")
            s_vec = stage.tile([P, N_D + N_H], f32, tag="s_vec")
            xt_all = stage.tile([P, COLS], f32, tag="xt_all")

            nc.sync.dma_start(out=xt_all[:], in_=xt_d[:, :])
            sink0 = scratch.tile([P, 1], f32, tag="sink")
            nc.vector.tensor_tensor(
                out=sink0[:], in0=xt_all[:, 0:1], in1=ones[:], op=Alu.add
            )

            def dve_absorb(src_ap):
                t = scratch.tile([P, 1], f32, tag="eabs")
                nc.vector.tensor_tensor(
                    out=t[:], in0=src_ap, in1=ones[:], op=Alu.add
                )

            HALF = COLS // 2
            parts = stage.tile([P, 2], f32, tag="parts")

            def epilogue_half(h):
                ca = HALF_CT[h]["A"]
                cv = HALF_CT[h]["V"]
                sa_lo = HALF_CT[0]["A"] if h else 0
                sv_lo = HALF_CT[0]["V"] if h else 0
                s_all = stage.tile([P, HALF], f32, tag=f"s_all{h}")
                nc.scalar.copy(s_all[:, 0:ca], s_acc[:, sa_lo : sa_lo + ca])
                nc.scalar.copy(
                    s_all[:, ca : ca + cv], s_vec[:, sv_lo : sv_lo + cv]
                )

                ln_s = stage.tile([P, HALF], f32, tag=f"ln_s{h}")
                nc.scalar.activation(out=ln_s[:], in_=s_all[:], func=Act.Ln)
                logpt = stage.tile([P, HALF], f32, tag=f"logpt{h}")
                dve_absorb(ln_s[:, 0:1])
                nc.vector.tensor_tensor(
                    out=logpt[:], in0=xt_all[:, h * HALF : (h + 1) * HALF],
                    in1=ln_s[:], op=Alu.subtract,
                )
                pt = stage.tile([P, HALF], f32, tag=f"pt{h}")
                nc.scalar.activation(out=pt[:], in_=logpt[:], func=Act.Exp)
                om = stage.tile([P, HALF], f32, tag=f"om{h}")  # 1 - pt
                nc.scalar.activation(
                    out=om[:], in_=pt[:], func=Act.Copy, scale=-1.0, bias=1.0
                )
                ln1m = stage.tile([P, HALF], f32, tag=f"ln1m{h}")
                nc.scalar.activation(out=ln1m[:], in_=om[:], func=Act.Ln)

                # gamma = g0 + sum_i (g[i+1]-g[i]) * [pt >= e_i].
                # Half 0 runs mid-stream: use the Pool engine (frees DVE).
                # Half 1 is the kernel tail: DVE's shorter op latency wins.
                eng = nc.gpsimd if h == 0 else nc.vector
                gamma = stage.tile([P, HALF], f32, tag=f"gamma{h}")
                eng.memset(gamma[:], gammas[0])
                mtmp = stage.tile([P, HALF], f32, tag=f"mtmp{h}")
                for i in range(NUM_BINS - 1):
                    dg = gammas[i + 1] - gammas[i]
                    eng.tensor_scalar(
                        out=mtmp[:], in0=pt[:], scalar1=edges[i], scalar2=dg,
                        op0=Alu.is_ge, op1=Alu.mult,
                    )
                    eng.tensor_tensor(
                        out=gamma[:], in0=gamma[:], in1=mtmp[:], op=Alu.add
                    )

                prod = stage.tile([P, HALF], f32, tag=f"prod{h}")
                dve_absorb(ln1m[:, 0:1])
                nc.vector.tensor_tensor(
                    out=prod[:], in0=gamma[:], in1=ln1m[:], op=Alu.mult
                )
                focal = stage.tile([P, HALF], f32, tag=f"focal{h}")
                nc.scalar.activation(out=focal[:], in_=prod[:], func=Act.Exp)
                contrib = stage.tile([P, HALF], f32, tag=f"contrib{h}")
                dve_absorb(focal[:, 0:1])
                nc.vector.tensor_tensor(
                    out=contrib[:], in0=focal[:], in1=logpt[:], op=Alu.mult
                )
                nc.vector.tensor_reduce(
                    out=parts[:, h : h + 1], in_=contrib[:],
                    axis=mybir.AxisListType.X, op=Alu.add, negate=True,
                )

            for k in range(CHUNKS):
                ch = ASSIGN[k * J : (k + 1) * J]
                na_k = ch.count("A")
                nb_k = J - na_k
                assert na_k <= 2 and nb_k <= 3, (k, ch)
                x8_t = x16_t = None
                if na_k:
                    x8_t = x8pool.tile([P, 2, C], fp8, tag="x8")
                    src = x8_d[a_start[k] : a_start[k + 1]].rearrange(
                        "a p c -> p a c")
                    nc.sync.dma_start(out=x8_t[:, 0:na_k, :], in_=src)
                if nb_k:
                    x16_t = x16pool.tile([P, 3, C], bf16, tag="x16")
                    src = x16_d[b_start[k] : b_start[k + 1]].rearrange(
                        "a p c -> p a c")
                    nc.sync.dma_start(out=x16_t[:, 0:nb_k, :], in_=src)
                ai = bi = 0
                for j in range(J):
                    col = k * J + j
                    e, m = ch[j], ENG_IDX[col]
                    if e == "A":
                        dump = epool.tile([P, C], bf16, tag="exp_out")
                        nc.scalar.activation(
                            out=dump[:], in_=x8_t[:, ai, :], func=Act.Exp,
                            accum_out=s_acc[:, m : m + 1],
                        )
                        ai += 1
                        continue
                    if e == "D":
                        e_t = ipoolD.tile([P, C], i16, tag="schD")
                        nc.vector.tensor_scalar(
                            out=e_t[:], in0=x16_t[:, bi, :],
                            scalar1=SCH_C1, scalar2=SCH_C2,
                            op0=Alu.mult, op1=Alu.add,
                        )
                        nc.vector.tensor_reduce(
                            out=s_vec[:, m : m + 1], in_=e_t[:].bitcast(bf16),
                            axis=mybir.AxisListType.X, op=Alu.add,
                        )
                    else:  # H: Pool schraudolph in place, DVE reduce
                        nc.gpsimd.tensor_scalar(
                            out=x16_t[:, bi, :].bitcast(i16),
                            in0=x16_t[:, bi, :],
                            scalar1=SCH_C1, scalar2=SCH_C2,
                            op0=Alu.mult, op1=Alu.add,
                        )
                        nc.vector.tensor_reduce(
                            out=s_vec[:, m : m + 1], in_=x16_t[:, bi, :],
                            axis=mybir.AxisListType.X, op=Alu.add,
                        )
                    bi += 1
                if k == CHUNKS // 2 - 1:
                    epilogue_half(0)

            epilogue_half(1)
            part = stage.tile([P, 1], f32, tag="part")
            nc.vector.tensor_tensor(
                out=part[:], in0=parts[:, 0:1], in1=parts[:, 1:2], op=Alu.add
            )
            nc.sync.dma_start(out=out_d[:, :], in_=part[:])

    if hw_fixups:
        apply_hw_fixups(nc, mybir)
        verify_single_wait(nc)
    return nc


def apply_hw_fixups(nc, mybir):
    # Strip redundant own-engine waits (in-order queues make them no-ops).
    own_prefix = {
        "EngineType.DVE": "DVE",
        "EngineType.Activation": "Activation",
        "EngineType.Pool": "Pool",
        "EngineType.PE": "PE",
        "EngineType.SP": "SP",
    }
    for blk in nc.m.functions[0].blocks:
        for ins in blk.instructions:
            si = getattr(ins, "sync_info", None)
            if si is None or type(ins).__name__ == "InstDMACopy":
                continue
            if len(si.on_wait) <= 1:
                continue
            pref = own_prefix.get(str(getattr(ins, "engine", "")), None)
            if pref is None:
                continue
            keep = [w for w in si.on_wait if not w.ant_name.startswith(pref + "_")]
            if len(keep) < len(si.on_wait):
                ins.sync_info = type(si)(on_wait=keep, on_update=list(si.on_update))

    # In-place H blocks leave two structural two-wait cases, each with a
    # transitive single-wait replacement:
    #  - Pool tensor_scalar {DVE WAR, DMA RAW}: the slot-recycling DMA it
    #    waits on already carried the DVE wait for the old tenant's
    #    readers, so DMA-complete implies the WAR. Keep the DMA wait.
    #  - DVE tensor_reduce {Pool RAW, DMA}: the Pool tensor_scalar it
    #    waits on itself waited on that DMA. Keep the Pool wait.
    for blk in nc.m.functions[0].blocks:
        for ins in blk.instructions:
            si = getattr(ins, "sync_info", None)
            if si is None or type(ins).__name__ == "InstDMACopy":
                continue
            if len(si.on_wait) <= 1:
                continue
            eng = str(getattr(ins, "engine", ""))
            names = [w.ant_name for w in si.on_wait]
            if (
                eng == "EngineType.Pool"
                and type(ins).__name__ == "InstTensorScalarPtr"
                and len(si.on_wait) == 2
                and any(n.startswith("DVE") for n in names)
                and any(n.startswith("DMA") for n in names)
            ):
                keep = [w for w in si.on_wait if w.ant_name.startswith("DMA")]
                ins.sync_info = type(si)(on_wait=keep, on_update=list(si.on_update))
            elif (
                eng == "EngineType.DVE"
                and type(ins).__name__ == "InstTensorReduce"
                and len(si.on_wait) == 2
                and any(n.startswith("Pool") for n in names)
                and any(n.startswith("DMA") for n in names)
            ):
                keep = [w for w in si.on_wait if w.ant_name.startswith("Pool")]
                ins.sync_info = type(si)(on_wait=keep, on_update=list(si.on_update))

    # walrus' DMA encoding holds a single sync wait. The last reader of a
    # stream slot is a DVE op for x16 (reduces run after the Pool
    # tensor_scalar) and an ACT op for x8, so that engine's wait implies
    # all readers and (through the reader's own DMA wait and the in-order
    # queue) the previous slot writer.
    for blk in nc.m.functions[0].blocks:
        for ins in blk.instructions:
            si = getattr(ins, "sync_info", None)
            if si is None or type(ins).__name__ != "InstDMACopy":
                continue
            if len(si.on_wait) <= 1:
                continue
            keep = [w for w in si.on_wait if w.ant_name.startswith("DVE")] or [
                w for w in si.on_wait if w.ant_name.startswith("Activation")
            ]
            assert len(keep) == 1, (ins.name, [w.ant_name for w in si.on_wait])
            ins.sync_info = type(si)(on_wait=keep, on_update=list(si.on_update))

    # Split multi-wait kernel-tail drains into single-wait chains.
    for blk in nc.m.functions[0].blocks:
        il = blk.instructions
        i = 0
        while i < len(il):
            ins = il[i]
            si = getattr(ins, "sync_info", None)
            if (
                si is not None
                and type(ins).__name__ == "InstDrain"
                and len(si.on_wait) > 1
            ):
                SyncInfo = type(si)
                waits = list(si.on_wait)
                for k, w in enumerate(waits[:-1]):
                    d = mybir.InstDrain(
                        name=f"{ins.name}-w{k}", ins=[], outs=[],
                        bass_is_fusable=False,
                    )
                    d.engine = ins.engine
                    d.sync_info = SyncInfo(on_wait=[w], on_update=[])
                    il.insert(i, d)
                    i += 1
                ins.sync_info = SyncInfo(
                    on_wait=[waits[-1]], on_update=list(si.on_update)
                )
            i += 1


def verify_single_wait(nc):
    """Build-time check of the walrus single-wait constraint."""
    bad = []
    for blk in nc.m.functions[0].blocks:
        for ins in blk.instructions:
            si = getattr(ins, "sync_info", None)
            if si is not None and len(si.on_wait) > 1:
                bad.append(
                    (ins.name, type(ins).__name__,
                     str(getattr(ins, "engine", "")),
                     [w.ant_name for w in si.on_wait])
                )
    assert not bad, f"multi-wait instructions after fixups: {bad}"


def make_in_maps(input, target):
    import ml_dtypes

    x = np.asarray(input, dtype=np.float32)
    t = np.asarray(target).astype(np.int64)
    xt = x[np.arange(N), t]

    a_cols = [c for c in range(COLS) if ASSIGN[c] == "A"]
    b_cols = [c for c in range(COLS) if ASSIGN[c] != "A"]

    in_maps = []
    for core in range(NCORES):
        xs = x[core * RPC : (core + 1) * RPC]
        blocks = xs.reshape(COLS, P, C)
        x8 = np.ascontiguousarray(blocks[a_cols]).astype(ml_dtypes.float8_e4m3)
        x16 = np.ascontiguousarray(blocks[b_cols]).astype(ml_dtypes.bfloat16)
        xts = np.zeros((P, COLS), np.float32)
        xt_core = xt[core * RPC : (core + 1) * RPC].reshape(COLS, P)
        for c in range(COLS):
            xts[:, epilogue_col(c)] = xt_core[c]
        in_maps.append({"x8": x8, "x16": x16, "xt": xts})
    return in_maps


def kernel(input, target, bin_edges, bin_gammas):
    global LAST_RESULT
    from concourse.bass_utils import run_bass_kernel_spmd

    nc = build_program(bin_edges, bin_gammas)
    in_maps = make_in_maps(input, target)
    trace = bool(os.environ.get("BASS_TRACE"))
    res = run_bass_kernel_spmd(nc, in_maps, list(range(NCORES)), trace=trace)
    LAST_RESULT = res
    total = np.float64(0.0)
    for r in res.results:
        total += r["out"].astype(np.float64).sum()
    return np.float32(total)
